# revision 9
# baseline (speedup 1.0000x reference)
import sys, os, math
sys.path.insert(0, "/opt/trn_rl_repo")
import numpy as np
import ml_dtypes

import concourse.bass as bass
import concourse.mybir as mybir
import concourse.tile as tile
from concourse import bacc
from concourse.bass_utils import run_bass_kernel_spmd

F16 = mybir.dt.float16
F8 = mybir.dt.float8e4
F32 = mybir.dt.float32
AF = mybir.ActivationFunctionType
ALU = mybir.AluOpType

D = 2048; S = 2048; H = 16; DH = 128; DF = 8192
EPS = 1.1920929e-07
NB = 16          # d-blocks of 128
SC = 4           # s-chunks of 512
bf = np.float16

# ALiBi-windowed attention: heads grouped so one program (slot windows below)
# covers every core; blocks beyond dist*slope>25 carry e^-19 relative mass.
GROUPS = [[9, 10, 4, 0], [11, 12, 5, 1], [13, 14, 6, 2], [15, 8, 7, 3]]
SLOTW = [16, 16, 8, 6]   # per-slot kb window (16 = full)

_NC = None
LAST_EXEC_NS = None


def _win(slot, qc):
    W = SLOTW[slot]
    if W >= NB:
        return 0, NB
    r = (W - 4) // 2
    return max(0, 4 * qc - r), min(NB, 4 * qc + 4 + r)


def _build():
    nc = bacc.Bacc("TRN2", target_bir_lowering=False, debug=False)

    dram = {}
    def din(name, shape, dt=F16):
        dram[name] = nc.dram_tensor(name, list(shape), dt, kind="ExternalInput").ap()
        return dram[name]
    def dout(name, shape, dt=F16):
        dram[name] = nc.dram_tensor(name, list(shape), dt, kind="ExternalOutput").ap()
        return dram[name]

    xT_d   = din("xT",   [NB, 128, S])            # xT[db,p,s] = x[g].T
    wqk_d  = din("wqk",  [8, 128, NB, 128])       # q(4 heads)+k(4 heads) lhsT tiles
    wv_d   = din("wv",   [128, NB, 512])          # v weights, rhs layout
    wg_d   = din("wg",   [4, 128, NB, 128])
    wup_d  = din("wup",  [32, 128, NB, 128])      # 16 u1-blocks then 16 u2-blocks
    wdw_d  = din("wdw",  [16, 128, NB, 128])      # [ob, p(f), fb, dout] merged halves
    wout_d = din("wout", [16, 128, 4, 128])       # [ob, p(c), cb, dout]
    auxL_d = din("auxL", [128, 2, S])             # rank-2 alibi lhsT rows (32-aligned bases)
    auxR_d = din("auxR", [128, 2, S])
    diag_d = din("diag", [128, 4, 4, 512])
    bqk_d  = din("bqk",  [128, 8], F32)
    bg_d   = din("bg",   [128, 4], F32)
    bup_d  = din("bup",  [128, 32], F32)
    bv_d   = din("bv",   [128, 4], F32)

    ao_d  = dout("attn_outT", [16, 128, S])       # [ob, p(dout), s]
    fa_d  = dout("ffn_T",    [16, 128, S])

    with tile.TileContext(nc) as tc:
        with tc.tile_pool(name="const", bufs=1) as constp, \
             tc.tile_pool(name="ev", bufs=1) as evp:

            ones128 = constp.tile([128, 128], F16)
            nc.vector.memset(ones128[:], 1.0)
            ones1 = constp.tile([1, 128], F16)
            nc.vector.memset(ones1[:], 1.0)
            ones8 = constp.tile([128, 2, 128], F8)
            nc.vector.memset(ones8[:], 1.0)
            epst = constp.tile([1, 1], F32)
            nc.vector.memset(epst[:], EPS)
            bqk = constp.tile([128, 8], F32)
            nc.gpsimd.dma_start(out=bqk[:], in_=bqk_d[:, :])
            bg = constp.tile([128, 4], F32)
            nc.gpsimd.dma_start(out=bg[:], in_=bg_d[:, :])
            bup = constp.tile([128, 32], F32)
            nc.gpsimd.dma_start(out=bup[:], in_=bup_d[:, :])
            bv = constp.tile([128, 4], F32)
            nc.gpsimd.dma_start(out=bv[:], in_=bv_d[:, :])
            # first FFN up-block weights resident (ev pool lives whole kernel,
            # so this prefetch is not blocked by pool-stack reuse)
            wup0 = evp.tile([128, 2, NB, 128], F16, tag="wup0")

            with tc.tile_pool(name="xn", bufs=1) as xnp:
                xn = xnp.tile([128, NB, S], F16)

                with tc.tile_pool(name="att", bufs=1) as attp:
                    auxL = attp.tile([128, 2, S], F16)
                    auxR = attp.tile([128, 2, S], F16)
                    diag = attp.tile([128, 4, 4, 512], F16)
                    qkT = attp.tile([128, 8, S], F16)
                    vsb = attp.tile([128, NB, 512], F16)
                    gateT = attp.tile([128, 4, S], F16)

                    with tc.tile_pool(name="s1qk", bufs=3) as wqkp:
                        # hoist qk-weight tiles so their DMAs overlap phase 0
                        wq_tiles = []
                        for cb in range(8):
                            w = wqkp.tile([128, NB, 128], F16, tag="w")
                            nc.gpsimd.dma_start(out=w[:], in_=wqk_d[cb, :, :, :])
                            wq_tiles.append(w)

                        # ---- phase 0: load x + RMSNorm (pipelined) ----
                        with tc.tile_pool(name="p0", bufs=1) as p0, \
                             tc.tile_pool(name="psB", bufs=1, space="PSUM") as psB:
                            ms = psB.tile([1, S], F32, tag="pA", padded_shape=[128, S])
                            for db in range(NB):
                                nc.sync.dma_start(out=xn[:, db, :], in_=xT_d[db, :, :])
                                xsq = p0.tile([128, S], F16, tag="xsq", bufs=2)
                                nc.vector.tensor_tensor(out=xsq[:], in0=xn[:, db, :],
                                                        in1=xn[:, db, :], op=ALU.mult)
                                for sc in range(SC):
                                    nc.tensor.matmul(out=ms[:, sc*512:(sc+1)*512],
                                                     lhsT=ones128[:, 0:1],
                                                     rhs=xsq[:, sc*512:(sc+1)*512],
                                                     start=(db == 0), stop=(db == NB - 1))
                            rs = psB.tile([1, S], F32, tag="pB", padded_shape=[128, S])
                            rq = psB.tile([1, S], F32, tag="pA")
                            rbc_ps = psB.tile([128, S], F32, tag="pB")
                            rb = p0.tile([1, S], F16)
                            rbc = p0.tile([128, S], F16)
                            for sc in range(SC):
                                sl = slice(sc*512, (sc+1)*512)
                                nc.scalar.activation(rs[:, sl], ms[:, sl], AF.Identity,
                                                     bias=epst[:], scale=1.0 / D)
                                nc.vector.reciprocal_approx_fast(out=rq[:, sl], in_=rs[:, sl])
                                nc.scalar.activation(rb[:, sl], rq[:, sl], AF.Sqrt)
                                nc.tensor.matmul(out=rbc_ps[:, sl], lhsT=ones1[:],
                                                 rhs=rb[:, sl], start=True, stop=True)
                                nc.scalar.activation(rbc[:, sl], rbc_ps[:, sl], AF.Copy)
                            for db in range(NB):
                                nc.vector.tensor_tensor(out=xn[:, db, :], in0=xn[:, db, :],
                                                        in1=rbc[:], op=ALU.mult)

                        # ---- phase 1: qkT + v + gate ----
                        with tc.tile_pool(name="s1v", bufs=1) as wvp, \
                             tc.tile_pool(name="ps1", bufs=4, space="PSUM") as ps:
                            for cb in range(8):
                                w = wq_tiles[cb]
                                for sc in range(SC):
                                    p = ps.tile([128, 512], F32, tag="mm")
                                    for db in range(NB):
                                        nc.tensor.matmul(out=p[:], lhsT=w[:, db, :],
                                                         rhs=xn[:, db, sc*512:(sc+1)*512],
                                                         start=(db == 0), stop=(db == NB - 1))
                                    nc.scalar.activation(qkT[:, cb, sc*512:(sc+1)*512], p[:],
                                                         AF.Identity, bias=bqk[:, cb:cb+1])
                            wvt = wvp.tile([128, NB, 512], F16)
                            nc.gpsimd.dma_start(out=wvt[:], in_=wv_d[:, :, :])
                            for sb in range(NB):
                                p = ps.tile([128, 512], F32, tag="mm")
                                for db in range(NB):
                                    nc.tensor.matmul(out=p[:], lhsT=xn[:, db, sb*128:(sb+1)*128],
                                                     rhs=wvt[:, db, :],
                                                     start=(db == 0), stop=(db == NB - 1))
                                nc.scalar.activation(vsb[:, sb, :], p[:], AF.Copy)
                            for cb in range(4):
                                w = wqkp.tile([128, NB, 128], F16, tag="w")
                                nc.gpsimd.dma_start(out=w[:], in_=wg_d[cb, :, :, :])
                                for sc in range(SC):
                                    p = ps.tile([128, 512], F32, tag="mm")
                                    for db in range(NB):
                                        nc.tensor.matmul(out=p[:], lhsT=w[:, db, :],
                                                         rhs=xn[:, db, sc*512:(sc+1)*512],
                                                         start=(db == 0), stop=(db == NB - 1))
                                    nc.scalar.activation(gateT[:, cb, sc*512:(sc+1)*512], p[:],
                                                         AF.Sigmoid, bias=bg[:, cb:cb+1])
                            # prefetch attention aux + first FFN weights (queued
                            # behind the gate weight DMAs on the gpsimd engine)
                            nc.gpsimd.dma_start(out=auxL[:], in_=auxL_d[:, :, :])
                            nc.gpsimd.dma_start(out=auxR[:], in_=auxR_d[:, :, :])
                            nc.gpsimd.dma_start(out=diag[:], in_=diag_d[:, :, :, :])
                            nc.gpsimd.dma_start(out=wup0[:, 0, :, :], in_=wup_d[0, :, :, :])
                            nc.gpsimd.dma_start(out=wup0[:, 1, :, :], in_=wup_d[16, :, :, :])

                    # ---- phase 3+4: attention, then out_proj ----
                    with tc.tile_pool(name="work", bufs=1) as attw:
                        wo_all = attw.tile([128, 16, 4, 128], F16)
                        for ob in range(16):
                            nc.gpsimd.dma_start(out=wo_all[:, ob, :, :],
                                                in_=wout_d[ob, :, :, :])
                        with tc.tile_pool(name="ps2", bufs=1, space="PSUM") as ps2, \
                             tc.tile_pool(name="psA", bufs=4, space="PSUM") as psA:
                            for qc in range(SC):
                                q0 = qc * 512
                                for h in range(4):
                                    lo, hi = _win(h, qc)
                                    ctx = ps2.tile([128, 512], F32, tag="ctx", bufs=2)
                                    lps = ps2.tile([128, 512], F32, tag="lps", bufs=2)
                                    n = hi - lo
                                    pair = None
                                    for kb in range(lo, hi):
                                        ki = kb - lo
                                        sps = psA.tile([128, 512], F32, tag="sc")
                                        is_diag = (kb // 4 == qc)
                                        nc.tensor.matmul(out=sps[:],
                                                         lhsT=qkT[:, 4 + h, kb*128:(kb+1)*128],
                                                         rhs=qkT[:, h, q0:q0+512],
                                                         start=True, stop=is_diag)
                                        if is_diag:
                                            nc.vector.tensor_tensor(
                                                out=sps[:], in0=sps[:],
                                                in1=diag[:, h, kb % 4, :], op=ALU.add)
                                        else:
                                            sg = 0 if kb < 4 * qc else 1
                                            i = h * 2 + sg
                                            bp = 32 * (i % 4)
                                            tl = i // 4
                                            nc.tensor.matmul(out=sps[:],
                                                             lhsT=auxL[bp:bp+2, tl, kb*128:(kb+1)*128],
                                                             rhs=auxR[bp:bp+2, tl, q0:q0+512],
                                                             start=False, stop=True,
                                                             tile_position=(bp, 0))
                                        if ki % 2 == 0:
                                            pair = attw.tile([128, 2, 512], F8, tag="probs", bufs=4)
                                        pb = pair[:, ki % 2, :]
                                        nc.scalar.activation(pb, sps[:], AF.Exp)
                                        if ki % 2 == 1:
                                            # fp8 DoubleRow: sums both kbs of the pair
                                            nc.tensor.matmul(out=lps[:], lhsT=ones8[:],
                                                             rhs=pair[:],
                                                             start=(ki == 1), stop=(ki >= n - 2),
                                                             perf_mode=mybir.MatmulPerfMode.DoubleRow)
                                        elif ki == n - 1:
                                            # odd leftover: normal MM, fp16 ones x fp8 probs
                                            nc.tensor.matmul(out=lps[:], lhsT=ones128[:],
                                                             rhs=pb,
                                                             start=(n == 1), stop=True)
                                        nc.tensor.matmul(out=ctx[:],
                                                         lhsT=vsb[:, kb, h*128:(h+1)*128],
                                                         rhs=pb,
                                                         start=(kb == lo), stop=(kb == hi - 1))
                                    rl = attw.tile([128, 512], F32, tag="rl", bufs=2)
                                    nc.vector.reciprocal_approx_fast(out=rl[:], in_=lps[:])
                                    t1 = attw.tile([128, 512], F16, tag="t1", bufs=2)
                                    nc.vector.tensor_tensor(out=t1[:], in0=ctx[:], in1=rl[:],
                                                            op=ALU.mult)
                                    nc.vector.scalar_tensor_tensor(
                                        out=gateT[:, h, q0:q0+512], in0=t1[:],
                                        scalar=bv[:, h:h+1], in1=gateT[:, h, q0:q0+512],
                                        op0=ALU.add, op1=ALU.mult)

                        with tc.tile_pool(name="ps4", bufs=4, space="PSUM") as ps:
                            for sc in range(SC):
                                for ob in range(16):
                                    p = ps.tile([128, 512], F32, tag="mm")
                                    for cb in range(4):
                                        nc.tensor.matmul(out=p[:], lhsT=wo_all[:, ob, cb, :],
                                                         rhs=gateT[:, cb, sc*512:(sc+1)*512],
                                                         start=(cb == 0), stop=(cb == 3))
                                    o = attw.tile([128, 512], F16, tag="oev", bufs=4)
                                    nc.scalar.activation(o[:], p[:], AF.Copy)
                                    nc.sync.dma_start(out=ao_d[ob, :, sc*512:(sc+1)*512], in_=o[:])

                # att pool closed; xn still live
                # ---- phase 5: FFN (merged halves) ----
                with tc.tile_pool(name="ff", bufs=1) as ffp, \
                     tc.tile_pool(name="s5", bufs=3) as wstr2, \
                     tc.tile_pool(name="ps5", bufs=4, space="PSUM") as ps:
                    hsb = ffp.tile([128, NB, S], F16)
                    for fb in range(NB):
                        u = [None, None]
                        for ui in range(2):
                            if fb == 0:
                                w = wup0[:, ui, :, :]
                            else:
                                wt = wstr2.tile([128, NB, 128], F16, tag="w")
                                nc.gpsimd.dma_start(out=wt[:], in_=wup_d[16 * ui + fb, :, :, :])
                                w = wt[:]
                            ut = ffp.tile([128, S], F16, tag=f"u{ui}", bufs=2)
                            for sc in range(SC):
                                p = ps.tile([128, 512], F32, tag="mm")
                                for db in range(NB):
                                    nc.tensor.matmul(out=p[:], lhsT=w[:, db, :] if fb else wup0[:, ui, db, :],
                                                     rhs=xn[:, db, sc*512:(sc+1)*512],
                                                     start=(db == 0), stop=(db == NB - 1))
                                func = AF.Silu if ui == 0 else AF.Identity
                                nc.scalar.activation(ut[:, sc*512:(sc+1)*512], p[:], func,
                                                     bias=bup[:, 16*ui+fb:16*ui+fb+1])
                            u[ui] = ut
                        nc.vector.tensor_tensor(out=hsb[:, fb, :], in0=u[0][:], in1=u[1][:],
                                                op=ALU.mult)
                    for ob in range(16):
                        w = wstr2.tile([128, NB, 128], F16, tag="wdw")
                        nc.gpsimd.dma_start(out=w[:], in_=wdw_d[ob, :, :, :])
                        for sc in range(SC):
                            p = ps.tile([128, 512], F32, tag="mm")
                            for fb in range(NB):
                                nc.tensor.matmul(out=p[:], lhsT=w[:, fb, :],
                                                 rhs=hsb[:, fb, sc*512:(sc+1)*512],
                                                 start=(fb == 0), stop=(fb == NB - 1))
                            o = evp.tile([128, 512], F16, tag="oev", bufs=6)
                            nc.scalar.activation(o[:], p[:], AF.Copy)
                            nc.sync.dma_start(out=fa_d[ob, :, sc*512:(sc+1)*512], in_=o[:])

    nc.compile()
    return nc


def _slopes():
    start = 2.0 ** (-8.0 / H)
    return np.array([start ** (i + 1) for i in range(H)], dtype=np.float32)


def _host_shard(inputs):
    x = np.asarray(inputs["x"], np.float32)
    rms_w = np.asarray(inputs["rms_w"], np.float32)
    qkv_w = np.asarray(inputs["qkv_w"], np.float32) * rms_w[:, None]
    qkv_b = np.asarray(inputs["qkv_b"], np.float32)
    up_w = np.asarray(inputs["up_w"], np.float32) * rms_w[:, None]
    up_b = np.asarray(inputs["up_b"], np.float32)
    dw_w = np.asarray(inputs["dw_w"], np.float32)
    gate_w = np.asarray(inputs["gate_w"], np.float32) * rms_w[:, None]
    gate_b = np.asarray(inputs["gate_b"], np.float32)
    out_w = np.asarray(inputs["out_w"], np.float32)
    slopes = np.asarray(inputs["alibi_slopes"], np.float32)
    sc = 1.0 / math.sqrt(DH)
    idx = np.arange(S, dtype=np.float32)

    in_maps = []
    for c in range(8):
        g, j = c // 4, c % 4
        hds = GROUPS[j]
        fc = slice(2048 * j, 2048 * j + 2048)

        def hcat(mat, base, axis=1):
            sls = [slice(base + h * DH, base + (h + 1) * DH) for h in hds]
            if axis == 1:
                return np.concatenate([mat[:, s] for s in sls], 1)
            return np.concatenate([mat[s] for s in sls], 0)

        wq = hcat(qkv_w, 0) * sc                              # [2048, 512]
        wk = hcat(qkv_w, 2048)
        wqk = np.concatenate([wq, wk], 1)                     # [2048,1024]
        wqk_h = wqk.reshape(NB, 128, 8, 128).transpose(2, 1, 0, 3).astype(bf)
        wv = hcat(qkv_w, 4096)
        wv_h = wv.reshape(NB, 128, 512).transpose(1, 0, 2).astype(bf)
        wg_h = hcat(gate_w, 0).reshape(NB, 128, 4, 128).transpose(2, 1, 0, 3).astype(bf)
        wup = np.concatenate([up_w[:, fc], up_w[:, DF + 2048*j: DF + 2048*j + 2048]], 1)
        wup_h = wup.reshape(NB, 128, 32, 128).transpose(2, 1, 0, 3).astype(bf)
        wdw_h = dw_w[fc, :].reshape(16, 128, 16, 128).transpose(2, 1, 0, 3).astype(bf)
        wout_rows = np.concatenate([out_w[h*DH:(h+1)*DH, :] for h in hds], 0)  # [512, 2048]
        wout_h = wout_rows.reshape(4, 128, 16, 128).transpose(2, 1, 0, 3).astype(bf)

        bq = hcat(qkv_b[None, :], 0)[0] * sc
        bk = hcat(qkv_b[None, :], 2048)[0]
        bqk_h = np.concatenate([bq, bk]).reshape(8, 128).T.astype(np.float32).copy()
        bg_h = hcat(gate_b[None, :], 0)[0].reshape(4, 128).T.astype(np.float32).copy()
        bup_h = np.concatenate([up_b[fc], up_b[DF + 2048*j: DF + 2048*j + 2048]]
                               ).reshape(32, 128).T.astype(np.float32).copy()
        bv_h = hcat(qkv_b[None, :], 4096)[0].reshape(4, 128).T.astype(np.float32).copy()

        auxL = np.zeros((128, 2, S), np.float32)
        auxR = np.zeros((128, 2, S), np.float32)
        dg = np.zeros((4, 4, 128, 512), np.float32)
        for t, hh in enumerate(hds):
            s = slopes[hh]
            for sg in range(2):                     # 0: q>k (upper), 1: q<k (lower)
                i = t * 2 + sg
                b, tl = 32 * (i % 4), i // 4
                sgn = 1.0 if sg == 0 else -1.0
                auxL[b + 0, tl] = 1.0
                auxL[b + 1, tl] = sgn * s * idx
                auxR[b + 0, tl] = -sgn * s * idx
                auxR[b + 1, tl] = 1.0
            for m in range(4):
                p = np.arange(128)[:, None]; dq = np.arange(512)[None, :]
                dg[t, m] = -s * np.abs(dq - 128 * m - p)

        xT_h = x[g].T.reshape(NB, 128, S).astype(bf)

        in_maps.append({
            "xT": np.ascontiguousarray(xT_h),
            "wqk": np.ascontiguousarray(wqk_h), "wv": np.ascontiguousarray(wv_h),
            "wg": np.ascontiguousarray(wg_h), "wup": np.ascontiguousarray(wup_h),
            "wdw": np.ascontiguousarray(wdw_h), "wout": np.ascontiguousarray(wout_h),
            "auxL": auxL.astype(bf), "auxR": auxR.astype(bf),
            "diag": np.ascontiguousarray(dg.transpose(2, 0, 1, 3)).astype(bf),
            "bqk": bqk_h, "bg": bg_h, "bup": bup_h, "bv": bv_h,
        })
    return in_maps


def kernel(**inputs):
    global _NC
    if _NC is None:
        _NC = _build()
    in_maps = _host_shard(inputs)
    trace = os.environ.get("BASS_KERNEL_TRACE") == "1"
    res = run_bass_kernel_spmd(_NC, in_maps, list(range(8)), trace=trace)
    global LAST_EXEC_NS
    LAST_EXEC_NS = res.exec_time_ns
    out_b = np.asarray(inputs["out_b"], np.float32)
    dw_b = np.asarray(inputs["dw_b"], np.float32)
    out = np.zeros((2, S, D), np.float32)
    for c in range(8):
        g = c // 4
        r = res.results[c]
        # attn_outT rows are per-head ctx dims permuted by GROUPS; out_proj
        # already projected them back to model dims, so plain sums work.
        for k in ("attn_outT", "ffn_T"):
            out[g] += r[k].astype(np.float32).reshape(D, S).T
    out += out_b + dw_b
    return out


# revision 10
# speedup vs baseline: 1.0349x; 1.0349x over previous
import sys, os, math
sys.path.insert(0, "/opt/trn_rl_repo")
import numpy as np
import ml_dtypes

import concourse.bass as bass
import concourse.mybir as mybir
import concourse.tile as tile
from concourse import bacc
from concourse.bass_utils import run_bass_kernel_spmd

F16 = mybir.dt.float16
F8 = mybir.dt.float8e4
F32 = mybir.dt.float32
AF = mybir.ActivationFunctionType
ALU = mybir.AluOpType

D = 2048; S = 2048; H = 16; DH = 128; DF = 8192
EPS = 1.1920929e-07
NB = 16          # d-blocks of 128
SC = 4           # s-chunks of 512
bf = np.float16

# ALiBi-windowed attention: heads grouped so one program (slot windows below)
# covers every core; blocks beyond dist*slope>25 carry e^-19 relative mass.
GROUPS = [[9, 10, 4, 0], [11, 12, 5, 1], [13, 14, 6, 2], [15, 8, 7, 3]]
SLOTW = [16, 16, 8, 6]   # per-slot kb window (16 = full)

_NC = None
LAST_EXEC_NS = None


def _win(slot, qc):
    W = SLOTW[slot]
    if W >= NB:
        return 0, NB
    r = (W - 4) // 2
    return max(0, 4 * qc - r), min(NB, 4 * qc + 4 + r)


def _build():
    nc = bacc.Bacc("TRN2", target_bir_lowering=False, debug=False)

    dram = {}
    def din(name, shape, dt=F16):
        dram[name] = nc.dram_tensor(name, list(shape), dt, kind="ExternalInput").ap()
        return dram[name]
    def dout(name, shape, dt=F16):
        dram[name] = nc.dram_tensor(name, list(shape), dt, kind="ExternalOutput").ap()
        return dram[name]

    xT_d   = din("xT",   [NB, 128, S])            # xT[db,p,s] = x[g].T
    wqk_d  = din("wqk",  [8, 128, NB, 128])       # q(4 heads)+k(4 heads) lhsT tiles
    wv_d   = din("wv",   [128, NB, 512])          # v weights, rhs layout
    wg_d   = din("wg",   [4, 128, NB, 128])
    wup_d  = din("wup",  [32, 128, NB, 128])      # 16 u1-blocks then 16 u2-blocks
    wdw_d  = din("wdw",  [16, 128, NB, 128])      # [ob, p(f), fb, dout] merged halves
    wout_d = din("wout", [16, 128, 4, 128])       # [ob, p(c), cb, dout]
    auxL_d = din("auxL", [128, 2, S])             # rank-2 alibi lhsT rows (32-aligned bases)
    auxR_d = din("auxR", [128, 2, S])
    diag_d = din("diag", [128, 4, 4, 512])
    bqk_d  = din("bqk",  [128, 8], F32)
    bg_d   = din("bg",   [128, 4], F32)
    bup_d  = din("bup",  [128, 32], F32)
    bv_d   = din("bv",   [128, 4], F32)

    ao_d  = dout("attn_outT", [16, 128, S])       # [ob, p(dout), s]
    fa_d  = dout("ffn_T",    [16, 128, S])

    with tile.TileContext(nc) as tc:
        with tc.tile_pool(name="const", bufs=1) as constp, \
             tc.tile_pool(name="ev", bufs=1) as evp:

            ones128 = constp.tile([128, 128], F16)
            nc.vector.memset(ones128[:], 1.0)
            ones1 = constp.tile([1, 128], F16)
            nc.vector.memset(ones1[:], 1.0)
            ones8 = constp.tile([128, 2, 128], F8)
            nc.vector.memset(ones8[:], 1.0)
            epst = constp.tile([1, 1], F32)
            nc.vector.memset(epst[:], EPS)
            bqk = constp.tile([128, 8], F32)
            nc.gpsimd.dma_start(out=bqk[:], in_=bqk_d[:, :])
            bg = constp.tile([128, 4], F32)
            nc.gpsimd.dma_start(out=bg[:], in_=bg_d[:, :])
            bup = constp.tile([128, 32], F32)
            nc.gpsimd.dma_start(out=bup[:], in_=bup_d[:, :])
            bv = constp.tile([128, 4], F32)
            nc.gpsimd.dma_start(out=bv[:], in_=bv_d[:, :])
            # first FFN up-block weights resident (ev pool lives whole kernel,
            # so this prefetch is not blocked by pool-stack reuse)
            wup0 = evp.tile([128, 2, NB, 128], F16, tag="wup0")

            with tc.tile_pool(name="xn", bufs=1) as xnp:
                xn = xnp.tile([128, NB, S], F16)

                with tc.tile_pool(name="att", bufs=1) as attp:
                    auxL = attp.tile([128, 2, S], F16)
                    auxR = attp.tile([128, 2, S], F16)
                    diag = attp.tile([128, 4, 4, 512], F16)
                    qkT = attp.tile([128, 8, S], F16)
                    vsb = attp.tile([128, NB, 512], F16)
                    gateT = attp.tile([128, 4, S], F16)

                    with tc.tile_pool(name="s1qk", bufs=3) as wqkp:
                        # hoist qk-weight tiles so their DMAs overlap phase 0
                        wq_tiles = []
                        for cb in range(8):
                            w = wqkp.tile([128, NB, 128], F16, tag="w")
                            nc.gpsimd.dma_start(out=w[:], in_=wqk_d[cb, :, :, :])
                            wq_tiles.append(w)

                        # ---- phase 0: load x + RMSNorm (pipelined) ----
                        with tc.tile_pool(name="p0", bufs=1) as p0, \
                             tc.tile_pool(name="psB", bufs=1, space="PSUM") as psB:
                            ms = psB.tile([1, S], F32, tag="pA", padded_shape=[128, S])
                            for db in range(NB):
                                nc.sync.dma_start(out=xn[:, db, :], in_=xT_d[db, :, :])
                                xsq = p0.tile([128, S], F16, tag="xsq", bufs=2)
                                nc.vector.tensor_tensor(out=xsq[:], in0=xn[:, db, :],
                                                        in1=xn[:, db, :], op=ALU.mult)
                                for sc in range(SC):
                                    nc.tensor.matmul(out=ms[:, sc*512:(sc+1)*512],
                                                     lhsT=ones128[:, 0:1],
                                                     rhs=xsq[:, sc*512:(sc+1)*512],
                                                     start=(db == 0), stop=(db == NB - 1))
                            rs = psB.tile([1, S], F32, tag="pB", padded_shape=[128, S])
                            rq = psB.tile([1, S], F32, tag="pA")
                            rbc_ps = psB.tile([128, S], F32, tag="pB")
                            rb = p0.tile([1, S], F16)
                            rbc = p0.tile([128, S], F16)
                            for sc in range(SC):
                                sl = slice(sc*512, (sc+1)*512)
                                nc.scalar.activation(rs[:, sl], ms[:, sl], AF.Identity,
                                                     bias=epst[:], scale=1.0 / D)
                                nc.vector.reciprocal_approx_fast(out=rq[:, sl], in_=rs[:, sl])
                                nc.scalar.activation(rb[:, sl], rq[:, sl], AF.Sqrt)
                                nc.tensor.matmul(out=rbc_ps[:, sl], lhsT=ones1[:],
                                                 rhs=rb[:, sl], start=True, stop=True)
                                nc.scalar.activation(rbc[:, sl], rbc_ps[:, sl], AF.Copy)
                            for db in range(NB):
                                nc.vector.tensor_tensor(out=xn[:, db, :], in0=xn[:, db, :],
                                                        in1=rbc[:], op=ALU.mult)

                        # ---- phase 1: qkT + v + gate ----
                        with tc.tile_pool(name="s1v", bufs=1) as wvp, \
                             tc.tile_pool(name="ps1", bufs=4, space="PSUM") as ps:
                            for cb in range(8):
                                w = wq_tiles[cb]
                                for sc in range(SC):
                                    p = ps.tile([128, 512], F32, tag="mm")
                                    for db in range(NB):
                                        nc.tensor.matmul(out=p[:], lhsT=w[:, db, :],
                                                         rhs=xn[:, db, sc*512:(sc+1)*512],
                                                         start=(db == 0), stop=(db == NB - 1))
                                    nc.scalar.activation(qkT[:, cb, sc*512:(sc+1)*512], p[:],
                                                         AF.Identity, bias=bqk[:, cb:cb+1])
                            wvt = wvp.tile([128, NB, 512], F16)
                            nc.gpsimd.dma_start(out=wvt[:], in_=wv_d[:, :, :])
                            for sb in range(NB):
                                p = ps.tile([128, 512], F32, tag="mm")
                                for db in range(NB):
                                    nc.tensor.matmul(out=p[:], lhsT=xn[:, db, sb*128:(sb+1)*128],
                                                     rhs=wvt[:, db, :],
                                                     start=(db == 0), stop=(db == NB - 1))
                                nc.scalar.activation(vsb[:, sb, :], p[:], AF.Copy)
                            for cb in range(4):
                                w = wqkp.tile([128, NB, 128], F16, tag="w")
                                nc.gpsimd.dma_start(out=w[:], in_=wg_d[cb, :, :, :])
                                for sc in range(SC):
                                    p = ps.tile([128, 512], F32, tag="mm")
                                    for db in range(NB):
                                        nc.tensor.matmul(out=p[:], lhsT=w[:, db, :],
                                                         rhs=xn[:, db, sc*512:(sc+1)*512],
                                                         start=(db == 0), stop=(db == NB - 1))
                                    nc.scalar.activation(gateT[:, cb, sc*512:(sc+1)*512], p[:],
                                                         AF.Sigmoid, bias=bg[:, cb:cb+1])
                            # prefetch attention aux + first FFN weights (queued
                            # behind the gate weight DMAs on the gpsimd engine)
                            nc.gpsimd.dma_start(out=auxL[:], in_=auxL_d[:, :, :])
                            nc.gpsimd.dma_start(out=auxR[:], in_=auxR_d[:, :, :])
                            nc.gpsimd.dma_start(out=diag[:], in_=diag_d[:, :, :, :])
                            nc.gpsimd.dma_start(out=wup0[:, 0, :, :], in_=wup_d[0, :, :, :])
                            nc.gpsimd.dma_start(out=wup0[:, 1, :, :], in_=wup_d[16, :, :, :])

                    # ---- phase 3+4: attention, then out_proj ----
                    with tc.tile_pool(name="work", bufs=1) as attw:
                        wo_all = attw.tile([128, 16, 4, 128], F16)
                        for ob in range(16):
                            nc.gpsimd.dma_start(out=wo_all[:, ob, :, :],
                                                in_=wout_d[ob, :, :, :])
                        with tc.tile_pool(name="ps2", bufs=1, space="PSUM") as ps2, \
                             tc.tile_pool(name="psA", bufs=4, space="PSUM") as psA:
                            for qc in range(SC):
                                q0 = qc * 512
                                for h in range(4):
                                    lo, hi = _win(h, qc)
                                    ctx = ps2.tile([128, 512], F32, tag="ctx", bufs=2)
                                    lps = ps2.tile([128, 512], F32, tag="lps", bufs=2)
                                    for kb in range(lo, hi):
                                        sps = psA.tile([128, 512], F32, tag="sc")
                                        is_diag = (kb // 4 == qc)
                                        nc.tensor.matmul(out=sps[:],
                                                         lhsT=qkT[:, 4 + h, kb*128:(kb+1)*128],
                                                         rhs=qkT[:, h, q0:q0+512],
                                                         start=True, stop=is_diag)
                                        if is_diag:
                                            nc.vector.tensor_tensor(
                                                out=sps[:], in0=sps[:],
                                                in1=diag[:, h, kb % 4, :], op=ALU.add)
                                        else:
                                            sg = 0 if kb < 4 * qc else 1
                                            i = h * 2 + sg
                                            bp = 32 * (i % 4)
                                            tl = i // 4
                                            nc.tensor.matmul(out=sps[:],
                                                             lhsT=auxL[bp:bp+2, tl, kb*128:(kb+1)*128],
                                                             rhs=auxR[bp:bp+2, tl, q0:q0+512],
                                                             start=False, stop=True,
                                                             tile_position=(bp, 0))
                                        probs = attw.tile([128, 512], F16, tag="probs", bufs=6)
                                        nc.scalar.activation(probs[:], sps[:], AF.Exp)
                                        nc.tensor.matmul(out=lps[:], lhsT=ones128[:],
                                                         rhs=probs[:],
                                                         start=(kb == lo), stop=(kb == hi - 1))
                                        nc.tensor.matmul(out=ctx[:],
                                                         lhsT=vsb[:, kb, h*128:(h+1)*128],
                                                         rhs=probs[:],
                                                         start=(kb == lo), stop=(kb == hi - 1))
                                    rl = attw.tile([128, 512], F32, tag="rl", bufs=2)
                                    nc.vector.reciprocal_approx_fast(out=rl[:], in_=lps[:])
                                    t1 = attw.tile([128, 512], F16, tag="t1", bufs=2)
                                    nc.vector.tensor_tensor(out=t1[:], in0=ctx[:], in1=rl[:],
                                                            op=ALU.mult)
                                    nc.vector.scalar_tensor_tensor(
                                        out=gateT[:, h, q0:q0+512], in0=t1[:],
                                        scalar=bv[:, h:h+1], in1=gateT[:, h, q0:q0+512],
                                        op0=ALU.add, op1=ALU.mult)

                        with tc.tile_pool(name="ps4", bufs=4, space="PSUM") as ps:
                            for sc in range(SC):
                                for ob in range(16):
                                    p = ps.tile([128, 512], F32, tag="mm")
                                    for cb in range(4):
                                        nc.tensor.matmul(out=p[:], lhsT=wo_all[:, ob, cb, :],
                                                         rhs=gateT[:, cb, sc*512:(sc+1)*512],
                                                         start=(cb == 0), stop=(cb == 3))
                                    o = attw.tile([128, 512], F16, tag="oev", bufs=4)
                                    nc.scalar.activation(o[:], p[:], AF.Copy)
                                    nc.sync.dma_start(out=ao_d[ob, :, sc*512:(sc+1)*512], in_=o[:])

                # att pool closed; xn still live
                # ---- phase 5: FFN (merged halves) ----
                with tc.tile_pool(name="ff", bufs=1) as ffp, \
                     tc.tile_pool(name="s5", bufs=3) as wstr2, \
                     tc.tile_pool(name="ps5", bufs=4, space="PSUM") as ps:
                    hsb = ffp.tile([128, NB, S], F16)
                    for fb in range(NB):
                        u = [None, None]
                        for ui in range(2):
                            if fb == 0:
                                w = wup0[:, ui, :, :]
                            else:
                                wt = wstr2.tile([128, NB, 128], F16, tag="w")
                                nc.gpsimd.dma_start(out=wt[:], in_=wup_d[16 * ui + fb, :, :, :])
                                w = wt[:]
                            ut = ffp.tile([128, S], F16, tag=f"u{ui}", bufs=2)
                            for sc in range(SC):
                                p = ps.tile([128, 512], F32, tag="mm")
                                for db in range(NB):
                                    nc.tensor.matmul(out=p[:], lhsT=w[:, db, :] if fb else wup0[:, ui, db, :],
                                                     rhs=xn[:, db, sc*512:(sc+1)*512],
                                                     start=(db == 0), stop=(db == NB - 1))
                                func = AF.Silu if ui == 0 else AF.Identity
                                nc.scalar.activation(ut[:, sc*512:(sc+1)*512], p[:], func,
                                                     bias=bup[:, 16*ui+fb:16*ui+fb+1])
                            u[ui] = ut
                        nc.vector.tensor_tensor(out=hsb[:, fb, :], in0=u[0][:], in1=u[1][:],
                                                op=ALU.mult)
                    for ob in range(16):
                        w = wstr2.tile([128, NB, 128], F16, tag="wdw")
                        nc.gpsimd.dma_start(out=w[:], in_=wdw_d[ob, :, :, :])
                        for sc in range(SC):
                            p = ps.tile([128, 512], F32, tag="mm")
                            for fb in range(NB):
                                nc.tensor.matmul(out=p[:], lhsT=w[:, fb, :],
                                                 rhs=hsb[:, fb, sc*512:(sc+1)*512],
                                                 start=(fb == 0), stop=(fb == NB - 1))
                            o = evp.tile([128, 512], F16, tag="oev", bufs=6)
                            nc.scalar.activation(o[:], p[:], AF.Copy)
                            nc.sync.dma_start(out=fa_d[ob, :, sc*512:(sc+1)*512], in_=o[:])

    nc.compile()
    return nc


def _slopes():
    start = 2.0 ** (-8.0 / H)
    return np.array([start ** (i + 1) for i in range(H)], dtype=np.float32)


def _host_shard(inputs):
    x = np.asarray(inputs["x"], np.float32)
    rms_w = np.asarray(inputs["rms_w"], np.float32)
    qkv_w = np.asarray(inputs["qkv_w"], np.float32) * rms_w[:, None]
    qkv_b = np.asarray(inputs["qkv_b"], np.float32)
    up_w = np.asarray(inputs["up_w"], np.float32) * rms_w[:, None]
    up_b = np.asarray(inputs["up_b"], np.float32)
    dw_w = np.asarray(inputs["dw_w"], np.float32)
    gate_w = np.asarray(inputs["gate_w"], np.float32) * rms_w[:, None]
    gate_b = np.asarray(inputs["gate_b"], np.float32)
    out_w = np.asarray(inputs["out_w"], np.float32)
    slopes = np.asarray(inputs["alibi_slopes"], np.float32)
    sc = 1.0 / math.sqrt(DH)
    idx = np.arange(S, dtype=np.float32)

    in_maps = []
    for c in range(8):
        g, j = c // 4, c % 4
        hds = GROUPS[j]
        fc = slice(2048 * j, 2048 * j + 2048)

        def hcat(mat, base, axis=1):
            sls = [slice(base + h * DH, base + (h + 1) * DH) for h in hds]
            if axis == 1:
                return np.concatenate([mat[:, s] for s in sls], 1)
            return np.concatenate([mat[s] for s in sls], 0)

        wq = hcat(qkv_w, 0) * sc                              # [2048, 512]
        wk = hcat(qkv_w, 2048)
        wqk = np.concatenate([wq, wk], 1)                     # [2048,1024]
        wqk_h = wqk.reshape(NB, 128, 8, 128).transpose(2, 1, 0, 3).astype(bf)
        wv = hcat(qkv_w, 4096)
        wv_h = wv.reshape(NB, 128, 512).transpose(1, 0, 2).astype(bf)
        wg_h = hcat(gate_w, 0).reshape(NB, 128, 4, 128).transpose(2, 1, 0, 3).astype(bf)
        wup = np.concatenate([up_w[:, fc], up_w[:, DF + 2048*j: DF + 2048*j + 2048]], 1)
        wup_h = wup.reshape(NB, 128, 32, 128).transpose(2, 1, 0, 3).astype(bf)
        wdw_h = dw_w[fc, :].reshape(16, 128, 16, 128).transpose(2, 1, 0, 3).astype(bf)
        wout_rows = np.concatenate([out_w[h*DH:(h+1)*DH, :] for h in hds], 0)  # [512, 2048]
        wout_h = wout_rows.reshape(4, 128, 16, 128).transpose(2, 1, 0, 3).astype(bf)

        bq = hcat(qkv_b[None, :], 0)[0] * sc
        bk = hcat(qkv_b[None, :], 2048)[0]
        bqk_h = np.concatenate([bq, bk]).reshape(8, 128).T.astype(np.float32).copy()
        bg_h = hcat(gate_b[None, :], 0)[0].reshape(4, 128).T.astype(np.float32).copy()
        bup_h = np.concatenate([up_b[fc], up_b[DF + 2048*j: DF + 2048*j + 2048]]
                               ).reshape(32, 128).T.astype(np.float32).copy()
        bv_h = hcat(qkv_b[None, :], 4096)[0].reshape(4, 128).T.astype(np.float32).copy()

        auxL = np.zeros((128, 2, S), np.float32)
        auxR = np.zeros((128, 2, S), np.float32)
        dg = np.zeros((4, 4, 128, 512), np.float32)
        for t, hh in enumerate(hds):
            s = slopes[hh]
            for sg in range(2):                     # 0: q>k (upper), 1: q<k (lower)
                i = t * 2 + sg
                b, tl = 32 * (i % 4), i // 4
                sgn = 1.0 if sg == 0 else -1.0
                auxL[b + 0, tl] = 1.0
                auxL[b + 1, tl] = sgn * s * idx
                auxR[b + 0, tl] = -sgn * s * idx
                auxR[b + 1, tl] = 1.0
            for m in range(4):
                p = np.arange(128)[:, None]; dq = np.arange(512)[None, :]
                dg[t, m] = -s * np.abs(dq - 128 * m - p)

        xT_h = x[g].T.reshape(NB, 128, S).astype(bf)

        in_maps.append({
            "xT": np.ascontiguousarray(xT_h),
            "wqk": np.ascontiguousarray(wqk_h), "wv": np.ascontiguousarray(wv_h),
            "wg": np.ascontiguousarray(wg_h), "wup": np.ascontiguousarray(wup_h),
            "wdw": np.ascontiguousarray(wdw_h), "wout": np.ascontiguousarray(wout_h),
            "auxL": auxL.astype(bf), "auxR": auxR.astype(bf),
            "diag": np.ascontiguousarray(dg.transpose(2, 0, 1, 3)).astype(bf),
            "bqk": bqk_h, "bg": bg_h, "bup": bup_h, "bv": bv_h,
        })
    return in_maps


def kernel(**inputs):
    global _NC
    if _NC is None:
        _NC = _build()
    in_maps = _host_shard(inputs)
    trace = os.environ.get("BASS_KERNEL_TRACE") == "1"
    res = run_bass_kernel_spmd(_NC, in_maps, list(range(8)), trace=trace)
    global LAST_EXEC_NS
    LAST_EXEC_NS = res.exec_time_ns
    out_b = np.asarray(inputs["out_b"], np.float32)
    dw_b = np.asarray(inputs["dw_b"], np.float32)
    out = np.zeros((2, S, D), np.float32)
    for c in range(8):
        g = c // 4
        r = res.results[c]
        # attn_outT rows are per-head ctx dims permuted by GROUPS; out_proj
        # already projected them back to model dims, so plain sums work.
        for k in ("attn_outT", "ffn_T"):
            out[g] += r[k].astype(np.float32).reshape(D, S).T
    out += out_b + dw_b
    return out


# revision 18
# speedup vs baseline: 1.0367x; 1.0017x over previous
import sys, os, math
sys.path.insert(0, "/opt/trn_rl_repo")
import numpy as np
import ml_dtypes

import concourse.bass as bass
import concourse.mybir as mybir
import concourse.tile as tile
from concourse import bacc
from concourse.bass_utils import run_bass_kernel_spmd

F16 = mybir.dt.float16
F8 = mybir.dt.float8e4
F32 = mybir.dt.float32
AF = mybir.ActivationFunctionType
ALU = mybir.AluOpType

D = 2048; S = 2048; H = 16; DH = 128; DF = 8192
EPS = 1.1920929e-07
NB = 16          # d-blocks of 128
SC = 4           # s-chunks of 512
bf = np.float16

# ALiBi-windowed attention: heads grouped so one program (slot windows below)
# covers every core; blocks beyond dist*slope>25 carry e^-19 relative mass.
GROUPS = [[9, 10, 4, 0], [11, 12, 5, 1], [13, 14, 6, 2], [15, 8, 7, 3]]
SLOTW = [16, 16, 8, 6]   # per-slot kb window (16 = full)

_NC = None
LAST_EXEC_NS = None


def _win(slot, qc):
    W = SLOTW[slot]
    if W >= NB:
        return 0, NB
    r = (W - 4) // 2
    return max(0, 4 * qc - r), min(NB, 4 * qc + 4 + r)


def _build():
    nc = bacc.Bacc("TRN2", target_bir_lowering=False, debug=False)

    dram = {}
    def din(name, shape, dt=F16):
        dram[name] = nc.dram_tensor(name, list(shape), dt, kind="ExternalInput").ap()
        return dram[name]
    def dout(name, shape, dt=F16):
        dram[name] = nc.dram_tensor(name, list(shape), dt, kind="ExternalOutput").ap()
        return dram[name]

    xT_d   = din("xT",   [NB, 128, S])            # xT[db,p,s] = x[g].T
    wqk_d  = din("wqk",  [8, 128, NB, 128])       # q(4 heads)+k(4 heads) lhsT tiles
    wv_d   = din("wv",   [128, NB, 512])          # v weights, rhs layout
    wg_d   = din("wg",   [4, 128, NB, 128])
    wup_d  = din("wup",  [32, 128, NB, 128])      # 16 u1-blocks then 16 u2-blocks
    wdw_d  = din("wdw",  [16, 128, NB, 128])      # [ob, p(f), fb, dout] merged halves
    wout_d = din("wout", [16, 128, 4, 128])       # [ob, p(c), cb, dout]
    auxL_d = din("auxL", [128, 2, S])             # rank-2 alibi lhsT rows (32-aligned bases)
    auxR_d = din("auxR", [128, 2, S])
    diag_d = din("diag", [128, 4, 4, 512])
    bqk_d  = din("bqk",  [128, 8], F32)
    bg_d   = din("bg",   [128, 4], F32)
    bup_d  = din("bup",  [128, 32], F32)
    bv_d   = din("bv",   [128, 4], F32)

    ao_d  = dout("attn_outT", [16, 128, S])       # [ob, p(dout), s]
    fa_d  = dout("ffn_T",    [16, 128, S])

    with tile.TileContext(nc) as tc:
        with tc.tile_pool(name="const", bufs=1) as constp, \
             tc.tile_pool(name="ev", bufs=1) as evp:

            ones128 = constp.tile([128, 128], F16)
            nc.vector.memset(ones128[:], 1.0)
            ones1 = constp.tile([1, 128], F16)
            nc.vector.memset(ones1[:], 1.0)
            ones8 = constp.tile([128, 2, 128], F8)
            nc.vector.memset(ones8[:], 1.0)
            epst = constp.tile([1, 1], F32)
            nc.vector.memset(epst[:], EPS)
            bqk = constp.tile([128, 8], F32)
            nc.gpsimd.dma_start(out=bqk[:], in_=bqk_d[:, :])
            bg = constp.tile([128, 4], F32)
            nc.gpsimd.dma_start(out=bg[:], in_=bg_d[:, :])
            bup = constp.tile([128, 32], F32)
            nc.gpsimd.dma_start(out=bup[:], in_=bup_d[:, :])
            bv = constp.tile([128, 4], F32)
            nc.gpsimd.dma_start(out=bv[:], in_=bv_d[:, :])
            # first FFN up-block weights resident (ev pool lives whole kernel,
            # so this prefetch is not blocked by pool-stack reuse)
            wup0 = evp.tile([128, 2, NB, 128], F16, tag="wup0")

            with tc.tile_pool(name="xn", bufs=1) as xnp:
                xn = xnp.tile([128, NB, S], F16)

                with tc.tile_pool(name="att", bufs=1) as attp:
                    auxL = attp.tile([128, 2, S], F16)
                    auxR = attp.tile([128, 2, S], F16)
                    diag = attp.tile([128, 4, 4, 512], F16)
                    qkT = attp.tile([128, 8, S], F16)
                    vsb = attp.tile([128, NB, 512], F16)
                    gateT = attp.tile([128, 4, S], F16)

                    with tc.tile_pool(name="s1qk", bufs=3) as wqkp:
                        # hoist qk-weight tiles so their DMAs overlap phase 0
                        wq_tiles = []
                        for cb in range(8):
                            w = wqkp.tile([128, NB, 128], F16, tag="w")
                            nc.gpsimd.dma_start(out=w[:], in_=wqk_d[cb, :, :, :])
                            wq_tiles.append(w)

                        # ---- phase 0: load x + RMSNorm (pipelined) ----
                        with tc.tile_pool(name="p0", bufs=1) as p0, \
                             tc.tile_pool(name="psB", bufs=1, space="PSUM") as psB:
                            ms = psB.tile([1, S], F32, tag="pA", padded_shape=[128, S])
                            for db in range(NB):
                                nc.sync.dma_start(out=xn[:, db, :], in_=xT_d[db, :, :])
                                xsq = p0.tile([128, S], F16, tag="xsq", bufs=2)
                                nc.vector.tensor_tensor(out=xsq[:], in0=xn[:, db, :],
                                                        in1=xn[:, db, :], op=ALU.mult)
                                for sc in range(SC):
                                    nc.tensor.matmul(out=ms[:, sc*512:(sc+1)*512],
                                                     lhsT=ones128[:, 0:1],
                                                     rhs=xsq[:, sc*512:(sc+1)*512],
                                                     start=(db == 0), stop=(db == NB - 1))
                            rs = psB.tile([1, S], F32, tag="pB", padded_shape=[128, S])
                            rq = psB.tile([1, S], F32, tag="pA")
                            rbc_ps = psB.tile([128, S], F32, tag="pB")
                            rb = p0.tile([1, S], F16)
                            rbc = p0.tile([128, S], F16)
                            for sc in range(SC):
                                sl = slice(sc*512, (sc+1)*512)
                                nc.scalar.activation(rs[:, sl], ms[:, sl], AF.Identity,
                                                     bias=epst[:], scale=1.0 / D)
                                nc.vector.reciprocal_approx_fast(out=rq[:, sl], in_=rs[:, sl])
                                nc.scalar.activation(rb[:, sl], rq[:, sl], AF.Sqrt)
                                nc.tensor.matmul(out=rbc_ps[:, sl], lhsT=ones1[:],
                                                 rhs=rb[:, sl], start=True, stop=True)
                                nc.scalar.activation(rbc[:, sl], rbc_ps[:, sl], AF.Copy)
                            for sc in range(SC):
                                sl = slice(sc*512, (sc+1)*512)
                                for db in range(NB):
                                    nc.vector.tensor_tensor(out=xn[:, db, sl],
                                                            in0=xn[:, db, sl],
                                                            in1=rbc[:, sl], op=ALU.mult)

                        # ---- phase 1: qkT + v + gate ----
                        with tc.tile_pool(name="s1v", bufs=1) as wvp, \
                             tc.tile_pool(name="ps1", bufs=4, space="PSUM") as ps:
                            for cb in range(8):
                                w = wq_tiles[cb]
                                for sc in range(SC):
                                    p = ps.tile([128, 512], F32, tag="mm")
                                    for db in range(NB):
                                        nc.tensor.matmul(out=p[:], lhsT=w[:, db, :],
                                                         rhs=xn[:, db, sc*512:(sc+1)*512],
                                                         start=(db == 0), stop=(db == NB - 1))
                                    nc.scalar.activation(qkT[:, cb, sc*512:(sc+1)*512], p[:],
                                                         AF.Identity, bias=bqk[:, cb:cb+1])
                            wvt = wvp.tile([128, NB, 512], F16)
                            nc.gpsimd.dma_start(out=wvt[:], in_=wv_d[:, :, :])
                            for sb in range(NB):
                                p = ps.tile([128, 512], F32, tag="mm")
                                for db in range(NB):
                                    nc.tensor.matmul(out=p[:], lhsT=xn[:, db, sb*128:(sb+1)*128],
                                                     rhs=wvt[:, db, :],
                                                     start=(db == 0), stop=(db == NB - 1))
                                nc.scalar.activation(vsb[:, sb, :], p[:], AF.Copy)
                            for cb in range(4):
                                w = wqkp.tile([128, NB, 128], F16, tag="w")
                                nc.gpsimd.dma_start(out=w[:], in_=wg_d[cb, :, :, :])
                                for sc in range(SC):
                                    p = ps.tile([128, 512], F32, tag="mm")
                                    for db in range(NB):
                                        nc.tensor.matmul(out=p[:], lhsT=w[:, db, :],
                                                         rhs=xn[:, db, sc*512:(sc+1)*512],
                                                         start=(db == 0), stop=(db == NB - 1))
                                    nc.scalar.activation(gateT[:, cb, sc*512:(sc+1)*512], p[:],
                                                         AF.Sigmoid, bias=bg[:, cb:cb+1])
                            # prefetch attention aux + first FFN weights (queued
                            # behind the gate weight DMAs on the gpsimd engine)
                            nc.gpsimd.dma_start(out=auxL[:], in_=auxL_d[:, :, :])
                            nc.gpsimd.dma_start(out=auxR[:], in_=auxR_d[:, :, :])
                            nc.gpsimd.dma_start(out=diag[:], in_=diag_d[:, :, :, :])
                            nc.gpsimd.dma_start(out=wup0[:, 0, :, :], in_=wup_d[0, :, :, :])
                            nc.gpsimd.dma_start(out=wup0[:, 1, :, :], in_=wup_d[16, :, :, :])

                    # ---- phase 3+4: attention, then out_proj ----
                    with tc.tile_pool(name="work", bufs=1) as attw:
                        wo_all = attw.tile([128, 16, 4, 128], F16)
                        for ob in range(16):
                            nc.gpsimd.dma_start(out=wo_all[:, ob, :, :],
                                                in_=wout_d[ob, :, :, :])
                        with tc.tile_pool(name="ps2", bufs=1, space="PSUM") as ps2, \
                             tc.tile_pool(name="psA", bufs=4, space="PSUM") as psA:
                            for qc in range(SC):
                                q0 = qc * 512
                                for h in range(4):
                                    lo, hi = _win(h, qc)
                                    ctx = ps2.tile([128, 512], F32, tag="ctx", bufs=2)
                                    lps = ps2.tile([128, 512], F32, tag="lps", bufs=2)
                                    for kb in range(lo, hi):
                                        sps = psA.tile([128, 512], F32, tag="sc")
                                        is_diag = (kb // 4 == qc)
                                        nc.tensor.matmul(out=sps[:],
                                                         lhsT=qkT[:, 4 + h, kb*128:(kb+1)*128],
                                                         rhs=qkT[:, h, q0:q0+512],
                                                         start=True, stop=is_diag)
                                        if is_diag:
                                            nc.vector.tensor_tensor(
                                                out=sps[:], in0=sps[:],
                                                in1=diag[:, h, kb % 4, :], op=ALU.add)
                                        else:
                                            sg = 0 if kb < 4 * qc else 1
                                            i = h * 2 + sg
                                            bp = 32 * (i % 4)
                                            tl = i // 4
                                            nc.tensor.matmul(out=sps[:],
                                                             lhsT=auxL[bp:bp+2, tl, kb*128:(kb+1)*128],
                                                             rhs=auxR[bp:bp+2, tl, q0:q0+512],
                                                             start=False, stop=True,
                                                             tile_position=(bp, 0))
                                        probs = attw.tile([128, 512], F16, tag="probs", bufs=6)
                                        nc.scalar.activation(probs[:], sps[:], AF.Exp)
                                        nc.tensor.matmul(out=lps[:], lhsT=ones128[:],
                                                         rhs=probs[:],
                                                         start=(kb == lo), stop=(kb == hi - 1))
                                        nc.tensor.matmul(out=ctx[:],
                                                         lhsT=vsb[:, kb, h*128:(h+1)*128],
                                                         rhs=probs[:],
                                                         start=(kb == lo), stop=(kb == hi - 1))
                                    rl = attw.tile([128, 512], F32, tag="rl", bufs=2)
                                    nc.vector.reciprocal_approx_fast(out=rl[:], in_=lps[:])
                                    t1 = attw.tile([128, 512], F16, tag="t1", bufs=2)
                                    nc.vector.tensor_tensor(out=t1[:], in0=ctx[:], in1=rl[:],
                                                            op=ALU.mult)
                                    nc.vector.scalar_tensor_tensor(
                                        out=gateT[:, h, q0:q0+512], in0=t1[:],
                                        scalar=bv[:, h:h+1], in1=gateT[:, h, q0:q0+512],
                                        op0=ALU.add, op1=ALU.mult)

                        with tc.tile_pool(name="ps4", bufs=4, space="PSUM") as ps:
                            for sc in range(SC):
                                for ob in range(16):
                                    p = ps.tile([128, 512], F32, tag="mm")
                                    for cb in range(4):
                                        nc.tensor.matmul(out=p[:], lhsT=wo_all[:, ob, cb, :],
                                                         rhs=gateT[:, cb, sc*512:(sc+1)*512],
                                                         start=(cb == 0), stop=(cb == 3))
                                    o = attw.tile([128, 512], F16, tag="oev", bufs=4)
                                    nc.scalar.activation(o[:], p[:], AF.Copy)
                                    nc.sync.dma_start(out=ao_d[ob, :, sc*512:(sc+1)*512], in_=o[:])

                # att pool closed; xn still live
                # ---- phase 5: FFN (merged halves) ----
                with tc.tile_pool(name="ff", bufs=1) as ffp, \
                     tc.tile_pool(name="s5", bufs=3) as wstr2, \
                     tc.tile_pool(name="ps5", bufs=4, space="PSUM") as ps:
                    hsb = ffp.tile([128, NB, S], F16)
                    for fb in range(NB):
                        u = [None, None]
                        for ui in range(2):
                            if fb == 0:
                                w = wup0[:, ui, :, :]
                            else:
                                wt = wstr2.tile([128, NB, 128], F16, tag="w")
                                nc.gpsimd.dma_start(out=wt[:], in_=wup_d[16 * ui + fb, :, :, :])
                                w = wt[:]
                            ut = ffp.tile([128, S], F16, tag=f"u{ui}", bufs=2)
                            for sc in range(SC):
                                p = ps.tile([128, 512], F32, tag="mm")
                                for db in range(NB):
                                    nc.tensor.matmul(out=p[:], lhsT=w[:, db, :] if fb else wup0[:, ui, db, :],
                                                     rhs=xn[:, db, sc*512:(sc+1)*512],
                                                     start=(db == 0), stop=(db == NB - 1))
                                func = AF.Silu if ui == 0 else AF.Identity
                                nc.scalar.activation(ut[:, sc*512:(sc+1)*512], p[:], func,
                                                     bias=bup[:, 16*ui+fb:16*ui+fb+1])
                            u[ui] = ut
                        nc.vector.tensor_tensor(out=hsb[:, fb, :], in0=u[0][:], in1=u[1][:],
                                                op=ALU.mult)
                    for ob in range(16):
                        w = wstr2.tile([128, NB, 128], F16, tag="wdw")
                        nc.gpsimd.dma_start(out=w[:], in_=wdw_d[ob, :, :, :])
                        for sc in range(SC):
                            p = ps.tile([128, 512], F32, tag="mm")
                            for fb in range(NB):
                                nc.tensor.matmul(out=p[:], lhsT=w[:, fb, :],
                                                 rhs=hsb[:, fb, sc*512:(sc+1)*512],
                                                 start=(fb == 0), stop=(fb == NB - 1))
                            o = evp.tile([128, 512], F16, tag="oev", bufs=6)
                            nc.scalar.activation(o[:], p[:], AF.Copy)
                            eng = (nc.sync, nc.gpsimd)[(ob * SC + sc) % 2]
                            eng.dma_start(out=fa_d[ob, :, sc*512:(sc+1)*512], in_=o[:])

    nc.compile()
    return nc


def _slopes():
    start = 2.0 ** (-8.0 / H)
    return np.array([start ** (i + 1) for i in range(H)], dtype=np.float32)


def _host_shard(inputs):
    x = np.asarray(inputs["x"], np.float32)
    rms_w = np.asarray(inputs["rms_w"], np.float32)
    qkv_w = np.asarray(inputs["qkv_w"], np.float32) * rms_w[:, None]
    qkv_b = np.asarray(inputs["qkv_b"], np.float32)
    up_w = np.asarray(inputs["up_w"], np.float32) * rms_w[:, None]
    up_b = np.asarray(inputs["up_b"], np.float32)
    dw_w = np.asarray(inputs["dw_w"], np.float32)
    gate_w = np.asarray(inputs["gate_w"], np.float32) * rms_w[:, None]
    gate_b = np.asarray(inputs["gate_b"], np.float32)
    out_w = np.asarray(inputs["out_w"], np.float32)
    slopes = np.asarray(inputs["alibi_slopes"], np.float32)
    sc = 1.0 / math.sqrt(DH)
    idx = np.arange(S, dtype=np.float32)

    in_maps = []
    for c in range(8):
        g, j = c // 4, c % 4
        hds = GROUPS[j]
        fc = slice(2048 * j, 2048 * j + 2048)

        def hcat(mat, base, axis=1):
            sls = [slice(base + h * DH, base + (h + 1) * DH) for h in hds]
            if axis == 1:
                return np.concatenate([mat[:, s] for s in sls], 1)
            return np.concatenate([mat[s] for s in sls], 0)

        wq = hcat(qkv_w, 0) * sc                              # [2048, 512]
        wk = hcat(qkv_w, 2048)
        wqk = np.concatenate([wq, wk], 1)                     # [2048,1024]
        wqk_h = wqk.reshape(NB, 128, 8, 128).transpose(2, 1, 0, 3).astype(bf)
        wv = hcat(qkv_w, 4096)
        wv_h = wv.reshape(NB, 128, 512).transpose(1, 0, 2).astype(bf)
        wg_h = hcat(gate_w, 0).reshape(NB, 128, 4, 128).transpose(2, 1, 0, 3).astype(bf)
        wup = np.concatenate([up_w[:, fc], up_w[:, DF + 2048*j: DF + 2048*j + 2048]], 1)
        wup_h = wup.reshape(NB, 128, 32, 128).transpose(2, 1, 0, 3).astype(bf)
        wdw_h = dw_w[fc, :].reshape(16, 128, 16, 128).transpose(2, 1, 0, 3).astype(bf)
        wout_rows = np.concatenate([out_w[h*DH:(h+1)*DH, :] for h in hds], 0)  # [512, 2048]
        wout_h = wout_rows.reshape(4, 128, 16, 128).transpose(2, 1, 0, 3).astype(bf)

        bq = hcat(qkv_b[None, :], 0)[0] * sc
        bk = hcat(qkv_b[None, :], 2048)[0]
        bqk_h = np.concatenate([bq, bk]).reshape(8, 128).T.astype(np.float32).copy()
        bg_h = hcat(gate_b[None, :], 0)[0].reshape(4, 128).T.astype(np.float32).copy()
        bup_h = np.concatenate([up_b[fc], up_b[DF + 2048*j: DF + 2048*j + 2048]]
                               ).reshape(32, 128).T.astype(np.float32).copy()
        bv_h = hcat(qkv_b[None, :], 4096)[0].reshape(4, 128).T.astype(np.float32).copy()

        auxL = np.zeros((128, 2, S), np.float32)
        auxR = np.zeros((128, 2, S), np.float32)
        dg = np.zeros((4, 4, 128, 512), np.float32)
        for t, hh in enumerate(hds):
            s = slopes[hh]
            for sg in range(2):                     # 0: q>k (upper), 1: q<k (lower)
                i = t * 2 + sg
                b, tl = 32 * (i % 4), i // 4
                sgn = 1.0 if sg == 0 else -1.0
                auxL[b + 0, tl] = 1.0
                auxL[b + 1, tl] = sgn * s * idx
                auxR[b + 0, tl] = -sgn * s * idx
                auxR[b + 1, tl] = 1.0
            for m in range(4):
                p = np.arange(128)[:, None]; dq = np.arange(512)[None, :]
                dg[t, m] = -s * np.abs(dq - 128 * m - p)

        xT_h = x[g].T.reshape(NB, 128, S).astype(bf)

        in_maps.append({
            "xT": np.ascontiguousarray(xT_h),
            "wqk": np.ascontiguousarray(wqk_h), "wv": np.ascontiguousarray(wv_h),
            "wg": np.ascontiguousarray(wg_h), "wup": np.ascontiguousarray(wup_h),
            "wdw": np.ascontiguousarray(wdw_h), "wout": np.ascontiguousarray(wout_h),
            "auxL": auxL.astype(bf), "auxR": auxR.astype(bf),
            "diag": np.ascontiguousarray(dg.transpose(2, 0, 1, 3)).astype(bf),
            "bqk": bqk_h, "bg": bg_h, "bup": bup_h, "bv": bv_h,
        })
    return in_maps


def kernel(**inputs):
    global _NC
    if _NC is None:
        _NC = _build()
    in_maps = _host_shard(inputs)
    trace = os.environ.get("BASS_KERNEL_TRACE") == "1"
    res = run_bass_kernel_spmd(_NC, in_maps, list(range(8)), trace=trace)
    global LAST_EXEC_NS
    LAST_EXEC_NS = res.exec_time_ns
    out_b = np.asarray(inputs["out_b"], np.float32)
    dw_b = np.asarray(inputs["dw_b"], np.float32)
    out = np.zeros((2, S, D), np.float32)
    for c in range(8):
        g = c // 4
        r = res.results[c]
        # attn_outT rows are per-head ctx dims permuted by GROUPS; out_proj
        # already projected them back to model dims, so plain sums work.
        for k in ("attn_outT", "ffn_T"):
            out[g] += r[k].astype(np.float32).reshape(D, S).T
    out += out_b + dw_b
    return out
